# revision 1
# baseline (speedup 1.0000x reference)
"""DigitCapsuleLayer (dynamic routing) Trainium2 Bass kernel.

Sharding: P-parallel — the 1152 primary capsules are split 144-per-core
across 8 cores; every core holds the full batch B=128 on SBUF partitions.

Per core:
  phase 1 (TensorE): per p, u_hat[b, od] = x_p[8,128].T @ W_p[8,160]
    (K=8 contraction over in_dim), evacuated 3-p-at-a-time from PSUM into
    an SBUF-resident u_hat [128, 144*160].  A second accumulating matmul
    per p builds s1 = sum_p u_hat directly in PSUM (iter-1 coupling coeffs
    are uniform 1/10).
  routing iters (VectorE/ScalarE): softmax over o, weighted p-reduction,
    squash, b-update — all as [128, 23040] strided mul/reduce ops.
  cross-core: AllReduce (gpsimd collective) of the [128,160] partial s for
    iters 1 and 2; iter 3's partial s is returned and reduced on the host.
"""

import sys

sys.path.insert(0, "/opt/trn_rl_repo")

import numpy as np

B, P, IN_D, O, D = 128, 1152, 8, 10, 16
OD = O * D           # 160
NCORES = 8
PLOC = P // NCORES   # 144
EPS = 1e-8

_CACHE = {}


def _build():
    import os
    from concourse import bass, bacc, tile, mybir

    no_cc = bool(os.environ.get("CAPS_NO_CC"))
    f32 = mybir.dt.float32
    nc = bacc.Bacc("TRN2", target_bir_lowering=False, debug=False,
                   num_devices=1 if no_cc else NCORES)

    xT_d = nc.dram_tensor("xT", [IN_D, PLOC, B], f32, kind="ExternalInput")
    wT_d = nc.dram_tensor("wT", [IN_D, PLOC, OD], f32, kind="ExternalInput")
    out_d = nc.dram_tensor("sp3", [B, OD], f32, kind="ExternalOutput")

    CHUNK = 24            # p's per input-DMA chunk
    EV = 3                # p's per PSUM bank / evacuation copy
    NBLK = PLOC // EV     # 48 evacuation blocks

    with tile.TileContext(nc) as tc:
        with (
            tc.tile_pool(name="persist", bufs=1) as pp,
            tc.tile_pool(name="dram", bufs=2, space="DRAM") as dp,
            tc.tile_pool(name="psum_ub", bufs=6, space="PSUM") as pub,
            tc.tile_pool(name="psum_s1", bufs=1, space="PSUM") as ps1,
        ):
            uhat = pp.tile([B, PLOC * OD], f32)       # 90 KB/partition
            uhat_f = uhat[:]
            uhat4 = uhat_f.rearrange("b (p o d) -> b p o d", p=PLOC, o=O, d=D)

            s1_ps = ps1.tile([B, OD], f32)

            # ---------------- phase 1: u_hat + s1 ----------------
            with tc.tile_pool(name="p1", bufs=2) as p1:
                for ch in range(PLOC // CHUNK):
                    xc = p1.tile([IN_D, CHUNK, B], f32, tag="xc")
                    wc = p1.tile([IN_D, CHUNK, OD], f32, tag="wc")
                    sl = slice(ch * CHUNK, (ch + 1) * CHUNK)
                    nc.sync.dma_start(xc[:], xT_d[:, sl, :])
                    nc.sync.dma_start(wc[:], wT_d[:, sl, :])
                    for blk in range(CHUNK // EV):
                        ub = pub.tile([B, EV * OD], f32, tag="ub")
                        for k in range(EV):
                            j = blk * EV + k
                            p_glob = ch * CHUNK + j
                            nc.tensor.matmul(
                                ub[:, k * OD:(k + 1) * OD],
                                xc[:, j, :], wc[:, j, :],
                                start=True, stop=True,
                            )
                            nc.tensor.matmul(
                                s1_ps[:], xc[:, j, :], wc[:, j, :],
                                start=(p_glob == 0), stop=(p_glob == PLOC - 1),
                                skip_group_check=True,
                            )
                        gblk = ch * (CHUNK // EV) + blk
                        dst = uhat_f[:, gblk * EV * OD:(gblk + 1) * EV * OD]
                        if gblk % 2 == 0:
                            nc.scalar.copy(dst, ub[:])
                        else:
                            nc.vector.tensor_copy(dst, ub[:])

            with tc.tile_pool(name="work", bufs=1) as wp:
                # ---------------- routing tiles ----------------
                tmp = wp.tile([B, PLOC * OD], f32)        # 90 KB/partition
                tmp4 = tmp[:].rearrange("b (p o d) -> b p o d", p=PLOC, o=O, d=D)
                tmp_pod = tmp[:].rearrange("b (p o d) -> b o d p", p=PLOC, o=O, d=D)

                b_route = wp.tile([B, PLOC * O], f32)
                br3 = b_route[:].rearrange("b (p o) -> b p o", p=PLOC, o=O)
                delta = wp.tile([B, PLOC * O], f32)
                eb = wp.tile([B, PLOC * O], f32)
                eb3 = eb[:].rearrange("b (p o) -> b p o", p=PLOC, o=O)
                # delta doubles as the exp-output buffer: softmax's use of it is
                # dead by the time bupd's reduce writes it, and vice versa.
                c_t = wp.tile([B, PLOC * O], f32)
                c3 = c_t[:].rearrange("b (p o) -> b p o", p=PLOC, o=O)
                mx = wp.tile([B, PLOC], f32)
                zs = wp.tile([B, PLOC], f32)
                rz = wp.tile([B, PLOC], f32)

                s_sb = wp.tile([B, OD], f32)
                s_full = wp.tile([B, OD], f32)
                sq = wp.tile([B, OD], f32)
                v_t = wp.tile([B, OD], f32)
                n2 = wp.tile([B, O], f32)
                rt = wp.tile([B, O], f32)
                a1 = wp.tile([B, O], f32)
                a2 = wp.tile([B, O], f32)
                den = wp.tile([B, O], f32)
                rec = wp.tile([B, O], f32)
                g_t = wp.tile([B, O], f32)

                AX = mybir.AxisListType.X

                def bcast(a, b_ap):
                    return bass.broadcast_tensor_aps(a, b_ap)

                def allreduce(src_ap, dst_ap):
                    if no_cc:
                        nc.vector.tensor_copy(dst_ap, src_ap)
                        return
                    cin = dp.tile([B, OD], f32, tag="cin")
                    cout = dp.tile([B, OD], f32, tag="cout", addr_space="Shared")
                    nc.sync.dma_start(cin[:], src_ap)
                    nc.gpsimd.collective_compute(
                        "AllReduce", mybir.AluOpType.add,
                        replica_groups=[list(range(NCORES))],
                        ins=[cin.opt()], outs=[cout.opt()],
                    )
                    nc.sync.dma_start(dst_ap, cout[:])

                def squash():
                    # v = (n2/(1+n2)) * s / (sqrt(n2)+eps), per (b, o)
                    nc.vector.tensor_mul(sq[:], s_full[:], s_full[:])
                    nc.vector.reduce_sum(
                        n2[:], sq[:].rearrange("b (o d) -> b o d", o=O, d=D), axis=AX)
                    nc.scalar.sqrt(rt[:], n2[:])
                    nc.vector.tensor_scalar_add(a1[:], n2[:], 1.0)
                    nc.vector.tensor_scalar_add(a2[:], rt[:], EPS)
                    nc.vector.tensor_mul(den[:], a1[:], a2[:])
                    nc.vector.reciprocal(rec[:], den[:])
                    nc.vector.tensor_mul(g_t[:], n2[:], rec[:])
                    sf3 = s_full[:].rearrange("b (o d) -> b o d", o=O, d=D)
                    v3 = v_t[:].rearrange("b (o d) -> b o d", o=O, d=D)
                    ga, gb = bcast(sf3, g_t[:].unsqueeze(-1))
                    nc.vector.tensor_mul(v3, ga, gb)

                PSPL = 96    # DVE takes p<PSPL, GPSIMD the rest

                def bupd(first):
                    # b_route += sum_d u_hat * v
                    va = v_t[:].rearrange("b (o d) -> b o d", o=O, d=D).unsqueeze(1)
                    ua, vb = bcast(uhat4, va)
                    nc.vector.tensor_mul(tmp4[:, :PSPL], ua[:, :PSPL], vb[:, :PSPL])
                    nc.gpsimd.tensor_mul(tmp4[:, PSPL:], ua[:, PSPL:], vb[:, PSPL:])
                    if first:
                        nc.vector.reduce_sum(b_route[:], tmp4, axis=AX)
                    else:
                        nc.vector.reduce_sum(delta[:], tmp4, axis=AX)
                        nc.vector.tensor_add(b_route[:], b_route[:], delta[:])

                def softmax():
                    # no max-subtraction: |b_route| is small enough that exp()
                    # cannot overflow fp32, and softmax is shift-invariant
                    nc.scalar.activation(delta[:], b_route[:],
                                         mybir.ActivationFunctionType.Exp)
                    d3 = delta[:].rearrange("b (p o) -> b p o", p=PLOC, o=O)
                    nc.vector.reduce_sum(zs[:], d3, axis=AX)
                    nc.vector.reciprocal(rz[:], zs[:])
                    ea, rb = bcast(d3, rz[:].unsqueeze(-1))
                    nc.vector.tensor_mul(c3, ea, rb)

                def weighted_s(dst_ap):
                    # dst = sum_p c * u_hat   (c broadcast over d)
                    ca = c3.unsqueeze(-1)
                    ua, cb = bcast(uhat4, ca)
                    nc.vector.tensor_mul(tmp4[:, :PSPL], ua[:, :PSPL], cb[:, :PSPL])
                    nc.gpsimd.tensor_mul(tmp4[:, PSPL:], ua[:, PSPL:], cb[:, PSPL:])
                    nc.vector.reduce_sum(
                        dst_ap.rearrange("b (o d) -> b o d", o=O, d=D),
                        tmp_pod, axis=AX)

                # ---------------- routing ----------------
                # iter 1: c uniform = 1/10
                nc.scalar.mul(s_sb[:], s1_ps[:], 0.1)
                allreduce(s_sb[:], s_full[:])
                squash()
                bupd(first=True)

                # iter 2
                softmax()
                weighted_s(s_sb[:])
                allreduce(s_sb[:], s_full[:])
                squash()
                bupd(first=False)

                # iter 3: partial s only; reduce + squash on host
                softmax()
                weighted_s(s_sb[:])
                nc.sync.dma_start(out_d[:], s_sb[:])

    nc.compile()
    return nc


def _get_nc():
    if "nc" not in _CACHE:
        _CACHE["nc"] = _build()
    return _CACHE["nc"]


def kernel(x: np.ndarray, W: np.ndarray) -> np.ndarray:
    import os
    from concourse.bass_utils import run_bass_kernel_spmd

    nc = _get_nc()
    trace = bool(os.environ.get("CAPS_TRACE"))
    x = np.ascontiguousarray(x, dtype=np.float32)
    W = np.ascontiguousarray(W, dtype=np.float32)

    in_maps = []
    for c in range(NCORES):
        sl = slice(c * PLOC, (c + 1) * PLOC)
        xT = np.ascontiguousarray(x[:, sl, :].transpose(2, 1, 0))      # [8,144,128]
        wT = np.ascontiguousarray(
            W[0, sl].reshape(PLOC, OD, IN_D).transpose(2, 0, 1))       # [8,144,160]
        in_maps.append({"xT": xT, "wT": wT})

    res = run_bass_kernel_spmd(nc, in_maps, list(range(NCORES)),
                               trace=trace,
                               tmpdir=os.environ.get("CAPS_TRACE_DIR"))
    if trace:
        print(f"HW exec time: {res.exec_time_ns} ns")
        _CACHE["last_result"] = res
    s = np.zeros((B, OD), dtype=np.float32)
    for c in range(NCORES):
        s += res.results[c]["sp3"]

    s = s.reshape(B, O, D)
    n2 = np.sum(s * s, axis=-1, keepdims=True, dtype=np.float32)
    norm = np.sqrt(n2)
    v = (n2 / (1.0 + n2)) * s / (norm + EPS)
    return v.astype(np.float32)



# revision 9
# speedup vs baseline: 2.0354x; 2.0354x over previous
"""DigitCapsuleLayer (dynamic routing) Trainium2 Bass kernel.

Sharding: P-parallel — the 1152 primary capsules are split 144-per-core
across 8 cores; every core holds the full batch B=128 on SBUF partitions.

Per core (all heavy data bf16):
  phase 1 (TensorE): 18 chunks of 8 p's. lhsT = x chunk [(8p,8i)=64, 128b]
    (stationary, bf16 -> FWL), rhs = block-diagonal W [(8p,8i), 8p*160]
    -> u_hat[b, (p,od)] in one K=64 matmul pair per chunk (N=512+512+256),
    plus one dense-W accumulating matmul per chunk building s1 = sum_p u_hat.
    PSUM is evacuated to TWO bf16 SBUF copies of u_hat: u_pod [b,(p o d)]
    (ScalarE+DVE split): p-major for the b-update, and u_odp [b,(o d p)]
    (p-innermost) for the weighted sum — each layout keeps the innermost
    axis of every big DVE tensor_tensor op unit-stride so the bf16 2x mode
    engages.
  routing (DVE): muls are bf16 tensor_tensor (2x); segment reductions are
    contiguous bf16 halving-tree adds (2x) instead of 1x tensor_reduce.
  cross-core: AllReduce (gpsimd collective) of the [128,160] fp32 partial s
    for iters 1 and 2; iter 3's partial s is returned and reduced on host.
"""

import sys

sys.path.insert(0, "/opt/trn_rl_repo")

import numpy as np
import ml_dtypes

BF16 = ml_dtypes.bfloat16

B, P, IN_D, O, D = 128, 1152, 8, 10, 16
OD = O * D           # 160
NCORES = 8
PLOC = P // NCORES   # 144
EPS = 1e-8

CH = 8               # p's per phase-1 chunk
NCH = PLOC // CH     # 18
KCH = CH * IN_D      # 64 contraction rows per chunk
NBD = CH * OD        # 1280 block-diag output cols per chunk

_CACHE = {}


def _build():
    import os
    from concourse import bass, bacc, tile, mybir

    no_cc = bool(os.environ.get("CAPS_NO_CC"))
    debug = bool(os.environ.get("CAPS_DEBUG"))
    f32 = mybir.dt.float32
    bf = mybir.dt.bfloat16
    nc = bacc.Bacc("TRN2", target_bir_lowering=False, debug=False,
                   num_devices=1 if no_cc else NCORES)

    xT_d = nc.dram_tensor("xT", [NCH, KCH, B], bf, kind="ExternalInput")
    wBD_d = nc.dram_tensor("wBD", [NCH, KCH, NBD], bf, kind="ExternalInput")
    wDN_d = nc.dram_tensor("wDN", [NCH, KCH, OD], bf, kind="ExternalInput")
    out_d = nc.dram_tensor("sp3", [B, OD], f32, kind="ExternalOutput")
    if debug:
        dbg_d = {
            "d_upod": nc.dram_tensor("d_upod", [B, PLOC * OD], bf,
                                     kind="ExternalOutput"),
            "d_uodp": nc.dram_tensor("d_uodp", [B, PLOC * OD], bf,
                                     kind="ExternalOutput"),
            "d_s1": nc.dram_tensor("d_s1", [B, OD], f32,
                                   kind="ExternalOutput"),
            "d_br1": nc.dram_tensor("d_br1", [B, PLOC * O], f32,
                                    kind="ExternalOutput"),
            "d_ct2": nc.dram_tensor("d_ct2", [B, O * PLOC], bf,
                                    kind="ExternalOutput"),
            "d_s2": nc.dram_tensor("d_s2", [B, OD], f32,
                                   kind="ExternalOutput"),
        }

    AXX = None

    with tile.TileContext(nc) as tc:
        with (
            tc.tile_pool(name="persist", bufs=1) as pp,
            tc.tile_pool(name="dram", bufs=2, space="DRAM") as dp,
            tc.tile_pool(name="psum_ub", bufs=2, space="PSUM") as pub,
            tc.tile_pool(name="psum_s1", bufs=1, space="PSUM") as ps1,
        ):
            u_pod = pp.tile([B, PLOC * OD], bf)       # 45 KB/partition
            u_odp = pp.tile([B, PLOC * OD], bf)       # 45 KB/partition
            upod_f = u_pod[:]
            uodp_f = u_odp[:]
            upod4 = upod_f.rearrange("b (p o d) -> b p o d", p=PLOC, o=O, d=D)
            uodp4 = uodp_f.rearrange("b (o d p) -> b o d p", o=O, d=D, p=PLOC)

            s1_ps = ps1.tile([B, OD], f32)

            # ---------------- phase 1: u_hat + s1 ----------------
            with tc.tile_pool(name="p1", bufs=2) as p1:
                for g in range(NCH):
                    xc = p1.tile([KCH, B], bf, tag="xc")
                    wbd = p1.tile([KCH, NBD], bf, tag="wbd")
                    wdn = p1.tile([KCH, OD], bf, tag="wdn")
                    nc.sync.dma_start(xc[:], xT_d[g])
                    nc.sync.dma_start(wbd[:], wBD_d[g])
                    nc.sync.dma_start(wdn[:], wDN_d[g])
                    # pad to 3 full PSUM banks so each 512-col matmul
                    # output slice stays within one bank in both buffers
                    ub = pub.tile([B, NBD], f32, tag="ub",
                                  padded_shape=[B, 1536])
                    for j, (n0, n1) in enumerate(((0, 512), (512, 1024),
                                                  (1024, 1280))):
                        nc.tensor.matmul(
                            ub[:, n0:n1], xc[:], wbd[:, n0:n1],
                            start=True, stop=True,
                        )
                    nc.tensor.matmul(
                        s1_ps[:], xc[:], wdn[:],
                        start=(g == 0), stop=(g == NCH - 1),
                        skip_group_check=True,
                    )
                    # evac: DVE -> p-major copy, ScalarE -> p-innermost copy
                    nc.vector.tensor_copy(
                        upod_f[:, g * NBD:(g + 1) * NBD], ub[:])
                    nc.scalar.copy(
                        uodp4[:, :, :, g * CH:(g + 1) * CH],
                        ub[:].rearrange("b (p o d) -> b o d p", p=CH, o=O, d=D))

            with tc.tile_pool(name="work", bufs=1) as wp:
                # ---------------- routing tiles ----------------
                tmp = wp.tile([B, PLOC * OD], bf)         # 45 KB/partition
                tmp_f = tmp[:]
                tmp4 = tmp_f.rearrange("b (p o d) -> b p o d", p=PLOC, o=O, d=D)
                tmp4o = tmp_f.rearrange("b (o d p) -> b o d p", o=O, d=D, p=PLOC)

                b_route = wp.tile([B, PLOC * O], f32)
                delta = wp.tile([B, PLOC * O], f32)
                e_t = wp.tile([B, PLOC * O], f32)
                e3 = e_t[:].rearrange("b (p o) -> b p o", p=PLOC, o=O)
                cT = wp.tile([B, O * PLOC], bf)
                cT3 = cT[:].rearrange("b (o p) -> b o p", o=O, p=PLOC)
                zs = wp.tile([B, PLOC], f32)
                rz = wp.tile([B, PLOC], f32)

                s_sb = wp.tile([B, OD], f32)
                s_full = wp.tile([B, OD], f32)
                sq = wp.tile([B, OD], f32)
                v_bf = wp.tile([B, OD], bf)
                n2 = wp.tile([B, O], f32)
                rt = wp.tile([B, O], f32)
                a1 = wp.tile([B, O], f32)
                a2 = wp.tile([B, O], f32)
                den = wp.tile([B, O], f32)
                rec = wp.tile([B, O], f32)
                g_t = wp.tile([B, O], f32)

                AX = mybir.AxisListType.X

                def bcast(a, b_ap):
                    return bass.broadcast_tensor_aps(a, b_ap)

                def allreduce(src_ap, dst_ap):
                    if no_cc:
                        nc.vector.tensor_copy(dst_ap, src_ap)
                        return
                    cin = dp.tile([B, OD], f32, tag="cin")
                    cout = dp.tile([B, OD], f32, tag="cout", addr_space="Shared")
                    nc.sync.dma_start(cin[:], src_ap)
                    nc.gpsimd.collective_compute(
                        "AllReduce", mybir.AluOpType.add,
                        replica_groups=[list(range(NCORES))],
                        ins=[cin.opt()], outs=[cout.opt()],
                    )
                    nc.sync.dma_start(dst_ap, cout[:])

                def squash():
                    # v = (n2/(1+n2)) * s / (sqrt(n2)+eps), per (b, o)
                    nc.vector.tensor_mul(sq[:], s_full[:], s_full[:])
                    nc.vector.reduce_sum(
                        n2[:], sq[:].rearrange("b (o d) -> b o d", o=O, d=D),
                        axis=AX)
                    nc.scalar.sqrt(rt[:], n2[:])
                    nc.vector.tensor_scalar_add(a1[:], n2[:], 1.0)
                    nc.vector.tensor_scalar_add(a2[:], rt[:], EPS)
                    nc.vector.tensor_mul(den[:], a1[:], a2[:])
                    nc.vector.reciprocal(rec[:], den[:])
                    nc.vector.tensor_mul(g_t[:], n2[:], rec[:])
                    sf3 = s_full[:].rearrange("b (o d) -> b o d", o=O, d=D)
                    v3 = v_bf[:].rearrange("b (o d) -> b o d", o=O, d=D)
                    ga, gb = bcast(sf3, g_t[:].unsqueeze(-1))
                    nc.vector.tensor_mul(v3, ga, gb)

                def bupd(first):
                    # delta[b,p,o] = sum_d u_pod[b,p,o,d] * v[b,o,d]
                    va = v_bf[:].rearrange("b (o d) -> b o d", o=O, d=D)
                    va = va.unsqueeze(1)
                    ua, vb = bcast(upod4, va)
                    nc.vector.tensor_mul(tmp4, ua, vb)
                    # halving tree over d (innermost, contiguous bf16 runs)
                    t3 = tmp_f.rearrange("b (po d) -> b po d", po=PLOC * O, d=D)
                    nc.vector.tensor_add(t3[:, :, 0:8], t3[:, :, 0:8],
                                         t3[:, :, 8:16])
                    nc.vector.tensor_add(t3[:, :, 0:4], t3[:, :, 0:4],
                                         t3[:, :, 4:8])
                    nc.vector.tensor_add(t3[:, :, 0:2], t3[:, :, 0:2],
                                         t3[:, :, 2:4])
                    dst = b_route[:] if first else delta[:]
                    nc.vector.tensor_add(
                        dst.unsqueeze(2), t3[:, :, 0:1], t3[:, :, 1:2])
                    if not first:
                        nc.vector.tensor_add(b_route[:], b_route[:], delta[:])

                def softmax():
                    # cT[b,o,p] = softmax_o(b_route)[b,p,o], bf16 o-major
                    nc.scalar.activation(e_t[:], b_route[:],
                                         mybir.ActivationFunctionType.Exp)
                    nc.vector.reduce_sum(zs[:], e3, axis=AX)
                    nc.vector.reciprocal(rz[:], zs[:])
                    ea = e3.transpose([0, 2, 1])      # [b, o, p] view
                    ra = rz[:].unsqueeze(1)           # [b, 1, p]
                    ea2, rb = bcast(ea, ra)
                    nc.vector.tensor_mul(cT3, ea2, rb)

                def weighted_s():
                    # tmp[b,o,d,p] = cT[b,o,p] * u_odp[b,o,d,p]; tree over p
                    ca = cT3.unsqueeze(2)             # [b, o, 1, p]
                    ua, cb = bcast(uodp4, ca)
                    nc.vector.tensor_mul(tmp4o, ua, cb)
                    # halving tree over p (innermost runs): 144->72->36->18->9
                    t3 = tmp_f.rearrange("b (od p) -> b od p", od=OD, p=PLOC)
                    for h in (72, 36, 18, 9):
                        nc.vector.tensor_add(t3[:, :, 0:h], t3[:, :, 0:h],
                                             t3[:, :, h:2 * h])
                    # 9 = 4+4+carry(col 8)
                    nc.vector.tensor_add(t3[:, :, 0:4], t3[:, :, 0:4],
                                         t3[:, :, 4:8])
                    nc.vector.tensor_add(t3[:, :, 0:2], t3[:, :, 0:2],
                                         t3[:, :, 2:4])
                    nc.vector.tensor_add(t3[:, :, 0:1], t3[:, :, 0:1],
                                         t3[:, :, 1:2])
                    nc.vector.tensor_add(t3[:, :, 0:1], t3[:, :, 0:1],
                                         t3[:, :, 8:9])
                    nc.vector.tensor_copy(s_sb[:].unsqueeze(2), t3[:, :, 0:1])

                # ---------------- routing ----------------
                # iter 1: c uniform = 1/10
                nc.scalar.mul(s_sb[:], s1_ps[:], 0.1)
                if debug:
                    nc.sync.dma_start(dbg_d["d_upod"][:], upod_f)
                    nc.sync.dma_start(dbg_d["d_uodp"][:], uodp_f)
                    nc.sync.dma_start(dbg_d["d_s1"][:], s_sb[:])
                allreduce(s_sb[:], s_full[:])
                squash()
                bupd(first=True)
                if debug:
                    nc.sync.dma_start(dbg_d["d_br1"][:], b_route[:])

                # iter 2
                softmax()
                weighted_s()
                if debug:
                    nc.sync.dma_start(dbg_d["d_ct2"][:], cT[:])
                    nc.sync.dma_start(dbg_d["d_s2"][:], s_sb[:])
                allreduce(s_sb[:], s_full[:])
                squash()
                bupd(first=False)

                # iter 3: partial s only; reduce + squash on host
                softmax()
                weighted_s()
                nc.sync.dma_start(out_d[:], s_sb[:])

    nc.compile()
    return nc


def _get_nc():
    if "nc" not in _CACHE:
        _CACHE["nc"] = _build()
    return _CACHE["nc"]


def _prep_core(x, W, c):
    sl = slice(c * PLOC, (c + 1) * PLOC)
    xs = x[:, sl, :]                                   # [B, 144, 8]
    Wod = W[0, sl].reshape(PLOC, OD, IN_D)             # [144, 160, 8]
    # lhsT chunks: [NCH, (CH p, 8 i), B]
    xT2 = np.ascontiguousarray(
        xs.transpose(1, 2, 0).reshape(NCH, KCH, B).astype(BF16))
    # dense W stack: [NCH, (CH p, 8 i), OD]
    Wt = Wod.transpose(0, 2, 1)                        # [144, 8, 160]
    wDN = np.ascontiguousarray(Wt.reshape(NCH, KCH, OD).astype(BF16))
    # block-diagonal W: [NCH, KCH, CH*OD]
    wBD = np.zeros((NCH, KCH, NBD), dtype=BF16)
    Wc = Wt.reshape(NCH, CH, IN_D, OD)
    for ps in range(CH):
        wBD[:, ps * IN_D:(ps + 1) * IN_D, ps * OD:(ps + 1) * OD] = Wc[:, ps]
    return {"xT": xT2, "wBD": wBD, "wDN": wDN}


def kernel(x: np.ndarray, W: np.ndarray) -> np.ndarray:
    import os
    from concourse.bass_utils import run_bass_kernel_spmd

    nc = _get_nc()
    trace = bool(os.environ.get("CAPS_TRACE"))
    x = np.ascontiguousarray(x, dtype=np.float32)
    W = np.ascontiguousarray(W, dtype=np.float32)

    in_maps = [_prep_core(x, W, c) for c in range(NCORES)]

    res = run_bass_kernel_spmd(nc, in_maps, list(range(NCORES)),
                               trace=trace,
                               tmpdir=os.environ.get("CAPS_TRACE_DIR"))
    if trace:
        print(f"HW exec time: {res.exec_time_ns} ns")
        _CACHE["last_result"] = res
    s = np.zeros((B, OD), dtype=np.float32)
    for c in range(NCORES):
        s += res.results[c]["sp3"]

    s = s.reshape(B, O, D)
    n2 = np.sum(s * s, axis=-1, keepdims=True, dtype=np.float32)
    norm = np.sqrt(n2)
    v = (n2 / (1.0 + n2)) * s / (norm + EPS)
    return v.astype(np.float32)


# revision 15
# speedup vs baseline: 2.1178x; 1.0405x over previous
"""DigitCapsuleLayer (dynamic routing) Trainium2 Bass kernel.

Sharding: P-parallel — the 1152 primary capsules are split 144-per-core
across 8 cores; every core holds the full batch B=128 on SBUF partitions.

Per core (all heavy data bf16):
  phase 1 (TensorE): 18 chunks of 8 p's. lhsT = x chunk [(8p,8i)=64, 128b]
    (stationary, bf16 -> FWL), rhs = block-diagonal W [(8p,8i), 8p*160]
    -> u_hat[b, (p,od)] in one K=64 matmul pair per chunk (N=512+512+256),
    plus one dense-W accumulating matmul per chunk building s1 = sum_p u_hat.
    PSUM is evacuated to TWO bf16 SBUF copies of u_hat: u_pod [b,(p o d)]
    (ScalarE+DVE split): p-major for the b-update, and u_odp [b,(o d p)]
    (p-innermost) for the weighted sum — each layout keeps the innermost
    axis of every big DVE tensor_tensor op unit-stride so the bf16 2x mode
    engages.
  routing (DVE): muls are bf16 tensor_tensor (2x); segment reductions are
    contiguous bf16 halving-tree adds (2x) instead of 1x tensor_reduce.
  cross-core: AllReduce (gpsimd collective) of the [128,160] fp32 partial s
    for iters 1 and 2; iter 3's partial s is returned and reduced on host.
"""

import sys

sys.path.insert(0, "/opt/trn_rl_repo")

import numpy as np
import ml_dtypes

BF16 = ml_dtypes.bfloat16

B, P, IN_D, O, D = 128, 1152, 8, 10, 16
OD = O * D           # 160
NCORES = 8
PLOC = P // NCORES   # 144
EPS = 1e-8

CH = 8               # p's per phase-1 chunk
NCH = PLOC // CH     # 18
KCH = CH * IN_D      # 64 contraction rows per chunk
NBD = CH * OD        # 1280 block-diag output cols per chunk

_CACHE = {}


def _build():
    import os
    from concourse import bass, bacc, tile, mybir

    no_cc = bool(os.environ.get("CAPS_NO_CC"))
    debug = bool(os.environ.get("CAPS_DEBUG"))
    f32 = mybir.dt.float32
    bf = mybir.dt.bfloat16
    nc = bacc.Bacc("TRN2", target_bir_lowering=False, debug=False,
                   num_devices=1 if no_cc else NCORES)

    xT_d = nc.dram_tensor("xT", [KCH, NCH * B], bf, kind="ExternalInput")
    wBD_d = nc.dram_tensor("wBD", [KCH, NCH * NBD], bf, kind="ExternalInput")
    wDN_d = nc.dram_tensor("wDN", [KCH, NCH * OD], bf, kind="ExternalInput")
    out_d = nc.dram_tensor("sp3", [B, OD], f32, kind="ExternalOutput")
    if debug:
        dbg_d = {
            "d_upod": nc.dram_tensor("d_upod", [B, PLOC * OD], bf,
                                     kind="ExternalOutput"),
            "d_uodp": nc.dram_tensor("d_uodp", [B, PLOC * OD], bf,
                                     kind="ExternalOutput"),
            "d_s1": nc.dram_tensor("d_s1", [B, OD], f32,
                                   kind="ExternalOutput"),
            "d_br1": nc.dram_tensor("d_br1", [B, PLOC * O], f32,
                                    kind="ExternalOutput"),
            "d_ct2": nc.dram_tensor("d_ct2", [B, O * PLOC], bf,
                                    kind="ExternalOutput"),
            "d_s2": nc.dram_tensor("d_s2", [B, OD], f32,
                                   kind="ExternalOutput"),
        }

    AXX = None

    with tile.TileContext(nc) as tc:
        with (
            tc.tile_pool(name="persist", bufs=1) as pp,
            tc.tile_pool(name="dram", bufs=2, space="DRAM") as dp,
            tc.tile_pool(name="psum_ub", bufs=2, space="PSUM") as pub,
            tc.tile_pool(name="psum_s1", bufs=1, space="PSUM") as ps1,
        ):
            u_pod = pp.tile([B, PLOC * OD], bf)       # 45 KB/partition
            u_odp = pp.tile([B, PLOC * OD], bf)       # 45 KB/partition
            upod_f = u_pod[:]
            uodp_f = u_odp[:]
            upod4 = upod_f.rearrange("b (p o d) -> b p o d", p=PLOC, o=O, d=D)
            uodp4 = uodp_f.rearrange("b (o d p) -> b o d p", o=O, d=D, p=PLOC)

            s1_ps = ps1.tile([B, OD], f32)

            # small persistent routing tiles needed across phase boundaries
            s_sb = pp.tile([B, OD], f32)
            s_full = pp.tile([B, OD], f32)

            def allreduce(src_ap, dst_ap):
                if no_cc:
                    nc.vector.tensor_copy(dst_ap, src_ap)
                    return
                cin = dp.tile([B, OD], f32, tag="cin")
                cout = dp.tile([B, OD], f32, tag="cout", addr_space="Shared")
                nc.sync.dma_start(cin[:], src_ap)
                nc.gpsimd.collective_compute(
                    "AllReduce", mybir.AluOpType.add,
                    replica_groups=[list(range(NCORES))],
                    ins=[cin.opt()], outs=[cout.opt()],
                )
                nc.sync.dma_start(dst_ap, cout[:])

            # ---------------- phase 1: u_hat + s1 ----------------
            with tc.tile_pool(name="p1", bufs=1) as p1:
                xall = p1.tile([KCH, NCH * B], bf)
                wdnall = p1.tile([KCH, NCH * OD], bf)
                wbdall = p1.tile([KCH, NCH * NBD], bf)
                nc.sync.dma_start(xall[:], xT_d[:])
                nc.sync.dma_start(wdnall[:], wDN_d[:])
                # wBD in thirds so chunk-0 matmuls start early
                third = 6 * NBD
                for t in range(3):
                    nc.sync.dma_start(wbdall[:, t * third:(t + 1) * third],
                                      wBD_d[:, t * third:(t + 1) * third])

                # s1 first: its AllReduce overlaps the rest of phase 1
                for g in range(NCH):
                    nc.tensor.matmul(
                        s1_ps[:], xall[:, g * B:(g + 1) * B],
                        wdnall[:, g * OD:(g + 1) * OD],
                        start=(g == 0), stop=(g == NCH - 1),
                        skip_group_check=True,
                    )
                nc.scalar.mul(s_sb[:], s1_ps[:], 0.1)
                if debug:
                    nc.sync.dma_start(dbg_d["d_s1"][:], s_sb[:])
                allreduce(s_sb[:], s_full[:])

                for g in range(NCH):
                    xg = xall[:, g * B:(g + 1) * B]
                    # pad to 3 full PSUM banks so each 512-col matmul
                    # output slice stays within one bank in both buffers
                    ub = pub.tile([B, NBD], f32, tag="ub",
                                  padded_shape=[B, 1536])
                    for n0, n1 in ((0, 512), (512, 1024), (1024, 1280)):
                        nc.tensor.matmul(
                            ub[:, n0:n1], xg,
                            wbdall[:, g * NBD + n0:g * NBD + n1],
                            start=True, stop=True,
                        )
                    # evac: DVE -> p-major copy, ScalarE -> p-innermost copy
                    # (a few p-major chunks go to ScalarE for balance)
                    if g % 6 == 5:
                        nc.scalar.copy(upod_f[:, g * NBD:(g + 1) * NBD], ub[:])
                    else:
                        nc.vector.tensor_copy(
                            upod_f[:, g * NBD:(g + 1) * NBD], ub[:])
                    nc.scalar.copy(
                        uodp4[:, :, :, g * CH:(g + 1) * CH],
                        ub[:].rearrange("b (p o d) -> b o d p", p=CH, o=O, d=D))

            with tc.tile_pool(name="work", bufs=1) as wp:
                # ---------------- routing tiles ----------------
                tmp = wp.tile([B, PLOC * OD], bf)         # 45 KB/partition
                tmp_f = tmp[:]
                tmp4 = tmp_f.rearrange("b (p o d) -> b p o d", p=PLOC, o=O, d=D)
                tmp4o = tmp_f.rearrange("b (o d p) -> b o d p", o=O, d=D, p=PLOC)

                b_route = wp.tile([B, PLOC * O], f32)
                delta = wp.tile([B, PLOC * O], f32)
                e_t = wp.tile([B, PLOC * O], f32)
                e3 = e_t[:].rearrange("b (p o) -> b p o", p=PLOC, o=O)
                cT = wp.tile([B, O * PLOC], bf)
                cT3 = cT[:].rearrange("b (o p) -> b o p", o=O, p=PLOC)
                zs = wp.tile([B, PLOC], f32)
                rz = wp.tile([B, PLOC], f32)

                sq = wp.tile([B, OD], f32)
                v_bf = wp.tile([B, OD], bf)
                n2 = wp.tile([B, O], f32)
                rt = wp.tile([B, O], f32)
                a1 = wp.tile([B, O], f32)
                a2 = wp.tile([B, O], f32)
                den = wp.tile([B, O], f32)
                rec = wp.tile([B, O], f32)
                g_t = wp.tile([B, O], f32)

                AX = mybir.AxisListType.X

                def bcast(a, b_ap):
                    return bass.broadcast_tensor_aps(a, b_ap)

                def squash():
                    # v = (n2/(1+n2)) * s / (sqrt(n2)+eps), per (b, o)
                    nc.vector.tensor_mul(sq[:], s_full[:], s_full[:])
                    nc.vector.reduce_sum(
                        n2[:], sq[:].rearrange("b (o d) -> b o d", o=O, d=D),
                        axis=AX)
                    nc.scalar.sqrt(rt[:], n2[:])
                    nc.vector.tensor_scalar_add(a1[:], n2[:], 1.0)
                    nc.vector.tensor_scalar_add(a2[:], rt[:], EPS)
                    nc.vector.tensor_mul(den[:], a1[:], a2[:])
                    nc.vector.reciprocal(rec[:], den[:])
                    nc.vector.tensor_mul(g_t[:], n2[:], rec[:])
                    sf3 = s_full[:].rearrange("b (o d) -> b o d", o=O, d=D)
                    v3 = v_bf[:].rearrange("b (o d) -> b o d", o=O, d=D)
                    ga, gb = bcast(sf3, g_t[:].unsqueeze(-1))
                    nc.vector.tensor_mul(v3, ga, gb)

                def bupd(first):
                    # delta[b,p,o] = sum_d u_pod[b,p,o,d] * v[b,o,d]
                    va = v_bf[:].rearrange("b (o d) -> b o d", o=O, d=D)
                    va = va.unsqueeze(1)
                    ua, vb = bcast(upod4, va)
                    nc.vector.tensor_mul(tmp4, ua, vb)
                    # halving tree over d (innermost, contiguous bf16 runs)
                    t3 = tmp_f.rearrange("b (po d) -> b po d", po=PLOC * O, d=D)
                    nc.vector.tensor_add(t3[:, :, 0:8], t3[:, :, 0:8],
                                         t3[:, :, 8:16])
                    nc.vector.tensor_add(t3[:, :, 0:4], t3[:, :, 0:4],
                                         t3[:, :, 4:8])
                    nc.vector.tensor_add(t3[:, :, 0:2], t3[:, :, 0:2],
                                         t3[:, :, 2:4])
                    dst = b_route[:] if first else delta[:]
                    nc.vector.tensor_add(
                        dst.unsqueeze(2), t3[:, :, 0:1], t3[:, :, 1:2])
                    if not first:
                        nc.vector.tensor_add(b_route[:], b_route[:], delta[:])

                def softmax():
                    # cT[b,o,p] = softmax_o(b_route)[b,p,o], bf16 o-major
                    nc.scalar.activation(e_t[:], b_route[:],
                                         mybir.ActivationFunctionType.Exp)
                    nc.vector.reduce_sum(zs[:], e3, axis=AX)
                    nc.vector.reciprocal(rz[:], zs[:])
                    ea = e3.transpose([0, 2, 1])      # [b, o, p] view
                    ra = rz[:].unsqueeze(1)           # [b, 1, p]
                    ea2, rb = bcast(ea, ra)
                    nc.vector.tensor_mul(cT3, ea2, rb)

                def weighted_s():
                    # tmp[b,o,d,p] = cT[b,o,p] * u_odp[b,o,d,p]; tree over p
                    ca = cT3.unsqueeze(2)             # [b, o, 1, p]
                    ua, cb = bcast(uodp4, ca)
                    nc.vector.tensor_mul(tmp4o, ua, cb)
                    # halving tree over p (innermost runs): 144->72->36->18->9
                    t3 = tmp_f.rearrange("b (od p) -> b od p", od=OD, p=PLOC)
                    for h in (72, 36, 18, 9):
                        nc.vector.tensor_add(t3[:, :, 0:h], t3[:, :, 0:h],
                                             t3[:, :, h:2 * h])
                    # 9 = 4+4+carry(col 8)
                    nc.vector.tensor_add(t3[:, :, 0:4], t3[:, :, 0:4],
                                         t3[:, :, 4:8])
                    nc.vector.tensor_add(t3[:, :, 0:2], t3[:, :, 0:2],
                                         t3[:, :, 2:4])
                    nc.vector.tensor_add(t3[:, :, 0:1], t3[:, :, 0:1],
                                         t3[:, :, 1:2])
                    nc.vector.tensor_add(t3[:, :, 0:1], t3[:, :, 0:1],
                                         t3[:, :, 8:9])
                    nc.vector.tensor_copy(s_sb[:].unsqueeze(2), t3[:, :, 0:1])

                # ---------------- routing ----------------
                # iter 1: c uniform = 1/10 (s1 scale + AllReduce emitted in
                # phase 1 so the collective overlaps the block matmuls)
                if debug:
                    nc.sync.dma_start(dbg_d["d_upod"][:], upod_f)
                    nc.sync.dma_start(dbg_d["d_uodp"][:], uodp_f)
                squash()
                bupd(first=True)
                if debug:
                    nc.sync.dma_start(dbg_d["d_br1"][:], b_route[:])

                # iter 2
                softmax()
                weighted_s()
                if debug:
                    nc.sync.dma_start(dbg_d["d_ct2"][:], cT[:])
                    nc.sync.dma_start(dbg_d["d_s2"][:], s_sb[:])
                allreduce(s_sb[:], s_full[:])
                squash()
                bupd(first=False)

                # iter 3: partial s only; reduce + squash on host
                softmax()
                weighted_s()
                nc.sync.dma_start(out_d[:], s_sb[:])

    nc.compile()
    return nc


def _get_nc():
    if "nc" not in _CACHE:
        _CACHE["nc"] = _build()
    return _CACHE["nc"]


def _prep_core(x, W, c):
    sl = slice(c * PLOC, (c + 1) * PLOC)
    xs = x[:, sl, :]                                   # [B, 144, 8]
    Wod = W[0, sl].reshape(PLOC, OD, IN_D)             # [144, 160, 8]
    # lhsT chunks, partition-major: [(CH p, 8 i) = KCH, NCH, B]
    xT2 = np.ascontiguousarray(
        xs.transpose(1, 2, 0).reshape(NCH, KCH, B).transpose(1, 0, 2)
        .reshape(KCH, NCH * B).astype(BF16))
    # dense W stack: [KCH, NCH, OD]
    Wt = Wod.transpose(0, 2, 1)                        # [144, 8, 160]
    wDN = np.ascontiguousarray(
        Wt.reshape(NCH, KCH, OD).transpose(1, 0, 2)
        .reshape(KCH, NCH * OD).astype(BF16))
    # block-diagonal W: [KCH, NCH, CH*OD]
    wBD = np.zeros((NCH, KCH, NBD), dtype=BF16)
    Wc = Wt.reshape(NCH, CH, IN_D, OD)
    for ps in range(CH):
        wBD[:, ps * IN_D:(ps + 1) * IN_D, ps * OD:(ps + 1) * OD] = Wc[:, ps]
    wBD = np.ascontiguousarray(
        wBD.transpose(1, 0, 2).reshape(KCH, NCH * NBD))
    return {"xT": xT2, "wBD": wBD, "wDN": wDN}


def kernel(x: np.ndarray, W: np.ndarray) -> np.ndarray:
    import os
    from concourse.bass_utils import run_bass_kernel_spmd

    nc = _get_nc()
    trace = bool(os.environ.get("CAPS_TRACE"))
    x = np.ascontiguousarray(x, dtype=np.float32)
    W = np.ascontiguousarray(W, dtype=np.float32)

    in_maps = [_prep_core(x, W, c) for c in range(NCORES)]

    res = run_bass_kernel_spmd(nc, in_maps, list(range(NCORES)),
                               trace=trace,
                               tmpdir=os.environ.get("CAPS_TRACE_DIR"))
    if trace:
        print(f"HW exec time: {res.exec_time_ns} ns")
        _CACHE["last_result"] = res
    s = np.zeros((B, OD), dtype=np.float32)
    for c in range(NCORES):
        s += res.results[c]["sp3"]

    s = s.reshape(B, O, D)
    n2 = np.sum(s * s, axis=-1, keepdims=True, dtype=np.float32)
    norm = np.sqrt(n2)
    v = (n2 / (1.0 + n2)) * s / (norm + EPS)
    return v.astype(np.float32)
